# revision 15
# baseline (speedup 1.0000x reference)
"""Multi-head dot-product attention (with per-head LayerNorm on q/k/v) on 8
Trainium2 NeuronCores.

Model: x[4, 2048, 1024], 16 heads x 64 dim, LN (no affine) applied per head to
q/k/v projections, softmax attention, output projection.

Sharding: core = (batch, query-half). Each core owns one batch and 1024 query
tokens; it computes k/v for the full 2048 keys of its batch (25% duplicated
work, zero collectives). Attention is invariant to key order, so the host
rotates tokens per core to make the program pure SPMD (queries are always
rows 0:1024 of the per-core input).

Device layout highlights:
 - host pre-transposes x to xT [dmodel, seq] and casts matmul operands to bf16
 - projections computed in natural [token, feature] orientation; LN stats along
   the free axis (per-head groups of 64); normalized q/k DMA-transposed (XBAR)
   into [head_dim, token] layout for the attention matmuls
 - scores computed as sT [key, query]; no max-subtraction needed (LN bounds
   scores to +-8); exp on ScalarE with the 1/sqrt(64) scale folded in
 - softmax denominator via a ones-column appended to v (pv matmul M=65);
   1/l applied with gpsimd partition_broadcast + fused PSUM-drain multiply
"""

import sys

for _p in ("/opt/trn_rl_repo",):
    if _p not in sys.path:
        sys.path.insert(0, _p)

import numpy as np
import ml_dtypes
from contextlib import ExitStack

import concourse.bass as bass
import concourse.bacc as bacc
import concourse.tile as tile
from concourse import mybir
from concourse import bass_utils

BF16 = ml_dtypes.bfloat16

B, S, DM = 4, 2048, 1024
H, HD = 16, 64
NCORES = 8
SQ = S // 2          # query tokens per core
NT_K = S // 128      # 16 token tiles for k/v
NT_Q = SQ // 128     # 8 token tiles for q
NIT = DM // 128      # 8 contraction tiles
NOC = DM // 512      # 2 output column chunks
QB = 512             # query block width in attention
NQB = SQ // QB       # 2
LN_EPS = 1e-5


def _build_program():
    nc = bacc.Bacc("TRN2", target_bir_lowering=False, debug=False)

    f32 = mybir.dt.float32
    bf16 = mybir.dt.bfloat16

    xT_d = nc.dram_tensor("xt", [DM, S], bf16, kind="ExternalInput").ap()
    w_d = {
        n: nc.dram_tensor(f"w{n}", [DM, DM], bf16, kind="ExternalInput").ap()
        for n in ("q", "k", "v", "o")
    }
    b_d = {
        n: nc.dram_tensor(f"b{n}", [1, DM], f32, kind="ExternalInput").ap()
        for n in ("q", "k", "v", "o")
    }
    out_d = nc.dram_tensor("out", [SQ, DM], f32, kind="ExternalOutput").ap()

    with ExitStack() as ctx:
        tc = ctx.enter_context(tile.TileContext(nc))

        consts = ctx.enter_context(tc.tile_pool(name="consts", bufs=1))
        xT_p = ctx.enter_context(tc.tile_pool(name="xT", bufs=1))
        w_p = ctx.enter_context(tc.tile_pool(name="w", bufs=1))
        qT_p = ctx.enter_context(tc.tile_pool(name="qT", bufs=1))
        kT_p = ctx.enter_context(tc.tile_pool(name="kT", bufs=1))
        vA_p = ctx.enter_context(tc.tile_pool(name="vA", bufs=1))
        aT_p = ctx.enter_context(tc.tile_pool(name="aT", bufs=1))
        stage_p = ctx.enter_context(tc.tile_pool(name="stage", bufs=2))
        stagebf_p = ctx.enter_context(tc.tile_pool(name="stagebf", bufs=2))
        sq_p = ctx.enter_context(tc.tile_pool(name="sq", bufs=2))
        stats_p = ctx.enter_context(tc.tile_pool(name="stats", bufs=4))
        probs_p = ctx.enter_context(tc.tile_pool(name="probs", bufs=4))
        rr_p = ctx.enter_context(tc.tile_pool(name="rr", bufs=2))
        rb_p = ctx.enter_context(tc.tile_pool(name="rb", bufs=2))
        outst_p = ctx.enter_context(tc.tile_pool(name="outst", bufs=2))

        psA = ctx.enter_context(tc.tile_pool(name="psA", bufs=2, space="PSUM"))
        psS = ctx.enter_context(tc.tile_pool(name="psS", bufs=4, space="PSUM"))
        psO = ctx.enter_context(tc.tile_pool(name="psO", bufs=2, space="PSUM"))
        dram_p = ctx.enter_context(tc.tile_pool(name="dram", bufs=4, space="DRAM"))

        # ---- persistent tiles ----
        xT = xT_p.tile([128, NIT, S], bf16)
        nc.sync.dma_start(out=xT, in_=xT_d.rearrange("(t p) s -> p t s", p=128))

        bias_t = {}
        for n in ("q", "k", "v", "o"):
            bt = consts.tile([128, DM], f32, tag=f"bias_{n}")
            bcast = bass.AP(
                tensor=b_d[n].tensor, offset=b_d[n].offset,
                ap=[[0, 128], b_d[n].ap[1]],
            )
            nc.gpsimd.dma_start(out=bt, in_=bcast)
            bias_t[n] = bt

        qT = qT_p.tile([128, NIT, SQ], bf16)    # [d-part, head-pair, q-token]
        kT = kT_p.tile([128, NIT, S], bf16)     # [d-part, head-pair, k-token]
        vA = vA_p.tile([128, NT_K, H, HD + 1], bf16)  # [k-part, ktile, head, d+1]
        aT = aT_p.tile([128, NIT, SQ], bf16)    # attn outT [d-part, head-pair, q]

        # ones column of v (softmax denominator rides along the pv matmul)
        nc.vector.memset(vA[:, :, :, HD:HD + 1], 1.0)

        eps_t = consts.tile([128, 1], f32, tag="eps")
        nc.vector.memset(eps_t, LN_EPS)

        # ---- projections + LN (+ transpose for q/k) ----
        def load_w(name):
            wt = w_p.tile([128, NIT, DM], bf16, tag="w")
            nc.sync.dma_start(
                out=wt, in_=w_d[name].rearrange("(t p) o -> p t o", p=128)
            )
            return wt

        def proj_ln(name, ntt):
            """Project (natural orientation), add bias, per-head LN."""
            wt = load_w(name)
            for tt in range(ntt):
                st = stage_p.tile([128, DM], f32, tag="stage")
                for oc in range(NOC):
                    ps = psA.tile([128, 512], f32, tag="psA")
                    for it in range(NIT):
                        nc.tensor.matmul(
                            ps,
                            xT[:, it, tt * 128:(tt + 1) * 128],
                            wt[:, it, oc * 512:(oc + 1) * 512],
                            start=(it == 0), stop=(it == NIT - 1),
                        )
                    nc.vector.tensor_add(
                        out=st[:, oc * 512:(oc + 1) * 512],
                        in0=ps,
                        in1=bias_t[name][:, oc * 512:(oc + 1) * 512],
                    )
                st3 = st.rearrange("p (h d) -> p h d", h=H)
                # stats
                sums = stats_p.tile([128, H], f32, tag="sums")
                nc.vector.tensor_reduce(
                    out=sums, in_=st3, axis=mybir.AxisListType.X,
                    op=mybir.AluOpType.add,
                )
                sqt = sq_p.tile([128, DM], f32, tag="sq")
                nc.scalar.square(out=sqt, in_=st)
                ssq = stats_p.tile([128, H], f32, tag="ssq")
                nc.vector.tensor_reduce(
                    out=ssq, in_=sqt.rearrange("p (h d) -> p h d", h=H),
                    axis=mybir.AxisListType.X, op=mybir.AluOpType.add,
                )
                mu = stats_p.tile([128, H], f32, tag="mu")
                nc.vector.tensor_scalar_mul(out=mu, in0=sums, scalar1=1.0 / HD)
                var = stats_p.tile([128, H], f32, tag="var")
                # var = ssq/HD - mu^2  (computed as (ssq*(1/HD)) - mu*mu)
                mu2 = stats_p.tile([128, H], f32, tag="mu2")
                nc.vector.tensor_mul(out=mu2, in0=mu, in1=mu)
                nc.vector.tensor_scalar_mul(out=var, in0=ssq, scalar1=1.0 / HD)
                nc.vector.tensor_sub(out=var, in0=var, in1=mu2)
                rstd = stats_p.tile([128, H], f32, tag="rstd")
                nc.scalar.activation(
                    out=rstd, in_=var, func=mybir.ActivationFunctionType.Sqrt,
                    bias=eps_t[:],
                )
                nc.vector.reciprocal(out=rstd, in_=rstd)

                def bcast3(t):
                    return bass.AP(
                        tensor=t.tensor, offset=t.offset,
                        ap=[t.ap[0], t.ap[1], [0, HD]],
                    )

                # center in place: st -= mu (st not needed raw after stats)
                nc.vector.tensor_sub(out=st3, in0=st3, in1=bcast3(mu))
                if name == "v":
                    nc.vector.tensor_mul(
                        out=vA[:, tt, :, 0:HD], in0=st3, in1=bcast3(rstd),
                    )
                else:
                    nb = stagebf_p.tile([128, DM], bf16, tag="nbf")
                    nc.vector.tensor_mul(
                        out=nb.rearrange("p (h d) -> p h d", h=H),
                        in0=st3, in1=bcast3(rstd),
                    )
                    dst = qT if name == "q" else kT
                    # one XBAR transpose for all 8 column blocks:
                    # dst[p, j, t] = nb[t, j*128+p]
                    nc.sync.dma_start_transpose(
                        dst[:, :, tt * 128:(tt + 1) * 128], nb,
                    )

        proj_ln("q", NT_Q)
        proj_ln("k", NT_K)
        proj_ln("v", NT_K)

        # ---- attention: per head pair, per query block ----
        for j in range(NIT):           # head pair (heads 2j, 2j+1)
            for qb in range(NQB):
                qsl = slice(qb * QB, (qb + 1) * QB)
                oP = [
                    psO.tile([HD + 1, QB], f32, tag="psO", name=f"oP{hh}")
                    for hh in range(2)
                ]
                for kt in range(NT_K):
                    ksl = slice(kt * 128, (kt + 1) * 128)
                    for hh in range(2):
                        psl = slice(hh * HD, (hh + 1) * HD)
                        sp = psS.tile([128, QB], f32, tag="psS")
                        nc.tensor.matmul(
                            sp, kT[psl, j, ksl], qT[psl, j, qsl],
                            start=True, stop=True,
                        )
                        pt = probs_p.tile([128, QB], bf16, tag="probs")
                        nc.scalar.activation(
                            out=pt, in_=sp,
                            func=mybir.ActivationFunctionType.Exp,
                            scale=1.0 / np.sqrt(HD),
                        )
                        nc.tensor.matmul(
                            oP[hh], vA[:, kt, 2 * j + hh, :], pt,
                            start=(kt == 0), stop=(kt == NT_K - 1),
                        )
                for hh in range(2):
                    rt = rr_p.tile([1, QB], f32, tag="rr")
                    nc.vector.reciprocal(out=rt, in_=oP[hh][HD:HD + 1, :])
                    # broadcast r along partitions via a DRAM bounce (DRAM
                    # source APs may have partition step 0; SBUF may not)
                    rd = dram_p.tile([1, QB], f32, tag="rd")
                    nc.sync.dma_start(out=rd, in_=rt)
                    rbt = rb_p.tile([HD, QB], f32, tag="rb")
                    nc.sync.dma_start(
                        out=rbt,
                        in_=bass.AP(tensor=rd.tensor, offset=rd.offset,
                                    ap=[[0, HD], rd.ap[1]]),
                    )
                    nc.vector.tensor_mul(
                        out=aT[hh * HD:(hh + 1) * HD, j, qsl],
                        in0=oP[hh][0:HD, :], in1=rbt,
                    )

        # ---- output projection ----
        wo = load_w("o")
        for tt in range(NT_Q):
            ot = outst_p.tile([128, DM], f32, tag="outst")
            for oc in range(NOC):
                ps = psA.tile([128, 512], f32, tag="psA")
                for j in range(NIT):
                    nc.tensor.matmul(
                        ps,
                        aT[:, j, tt * 128:(tt + 1) * 128],
                        wo[:, j, oc * 512:(oc + 1) * 512],
                        start=(j == 0), stop=(j == NIT - 1),
                    )
                nc.vector.tensor_add(
                    out=ot[:, oc * 512:(oc + 1) * 512],
                    in0=ps,
                    in1=bias_t["o"][:, oc * 512:(oc + 1) * 512],
                )
            nc.sync.dma_start(out=out_d[tt * 128:(tt + 1) * 128, :], in_=ot)

    nc.compile()
    return nc


_CACHE = {}


def _get_program():
    if "nc" not in _CACHE:
        _CACHE["nc"] = _build_program()
    return _CACHE["nc"]


def _make_in_maps(x, Wq, bq, Wk, bk, Wv, bv, Wo, bo):
    wq = np.ascontiguousarray(Wq.astype(BF16))
    wk = np.ascontiguousarray(Wk.astype(BF16))
    wv = np.ascontiguousarray(Wv.astype(BF16))
    wo = np.ascontiguousarray(Wo.astype(BF16))
    biases = {
        "bq": np.ascontiguousarray(bq.astype(np.float32).reshape(1, DM)),
        "bk": np.ascontiguousarray(bk.astype(np.float32).reshape(1, DM)),
        "bv": np.ascontiguousarray(bv.astype(np.float32).reshape(1, DM)),
        "bo": np.ascontiguousarray(bo.astype(np.float32).reshape(1, DM)),
    }
    in_maps = []
    for c in range(NCORES):
        b, hf = divmod(c, 2)
        xb = np.asarray(x[b])
        if hf:
            xb = np.concatenate([xb[SQ:], xb[:SQ]], axis=0)
        xt = np.ascontiguousarray(xb.T.astype(BF16))
        in_maps.append({
            "xt": xt, "wq": wq, "wk": wk, "wv": wv, "wo": wo, **biases,
        })
    return in_maps


def _run(x, Wq, bq, Wk, bk, Wv, bv, Wo, bo, **run_kwargs):
    nc = _get_program()
    in_maps = _make_in_maps(x, Wq, bq, Wk, bk, Wv, bv, Wo, bo)
    res = bass_utils.run_bass_kernel_spmd(
        nc, in_maps, core_ids=list(range(NCORES)), **run_kwargs
    )
    out = np.empty((B, S, DM), dtype=np.float32)
    for c in range(NCORES):
        b, hf = divmod(c, 2)
        out[b, hf * SQ:(hf + 1) * SQ] = res.results[c]["out"]
    return out, res


def kernel(x, Wq, bq, Wk, bk, Wv, bv, Wo, bo):
    out, _ = _run(x, Wq, bq, Wk, bk, Wv, bv, Wo, bo)
    return out


def kernel_profiled(x, Wq, bq, Wk, bk, Wv, bv, Wo, bo):
    return _run(x, Wq, bq, Wk, bk, Wv, bv, Wo, bo, trace=True)
